# revision 21
# baseline (speedup 1.0000x reference)
"""MultiHeadTEAttention TRN2 kernel — 8-core SPMD, one batch element per core.

v2.1 architecture (per core, batch m):
  - Kernel-MLP bias: R produced DIRECTLY in the (klo,c)-partition layout
    (fused add+relu tensor_scalar against per-partition b2 scalars; split
    DVE/GpSimd), contracted on PE with the fixed E-matrix
    E[(klo,c),(klo,h)] = kw2[c,h], exp'd on ACT straight out of PSUM.
  - Per-head un-shuffle of exp(bias) via 16 large strided DMAs per q-chunk
    (flat-AP partition-crossing), enabled by the per-k-tile permutation
    k~ = half*64 + klo*8 + pair (k = kt*128 + (2*pair+half)*8 + klo) applied
    consistently to k/v loads.
  - dots in [k,q] layout per head; one exp per side on ACT (free-1024
    batches); P = Et*Eb on DVE.
  - AV uses P-tiles as the matmul stationary with [v_h | ones] moving
    (65-wide free): attn-out lands in [q, hd] and the softmax denominator is
    the 65th column of the same accumulation — no separate ones-matmuls.
  - Software pipelining: E_b production for chunk qc+1 is interleaved with
    the per-head attention of chunk qc (and chunk 0 with the projections).
"""

import contextlib

import numpy as np

import concourse.bass as bass
import concourse.mybir as mybir
import concourse.tile as tile
from concourse import bacc, bass_utils

F32 = mybir.dt.float32
BF16 = mybir.dt.bfloat16
AX = mybir.AluOpType
AF = mybir.ActivationFunctionType

M, NQ, NKV, DX, DT = 8, 1024, 1024, 512, 2
H, HD = 8, 64
INNER = H * HD          # 512
KHID = 16               # c
SCALE = HD ** -0.5
P = 128
NKT = NKV // P          # 8 k-tiles
QC = 256                # q-chunk
NQC = NQ // QC          # 4
NIT = INNER // P        # 4 inner tiles
NDXT = DX // P          # 4 dx tiles


def build_kernel(nc: bass.Bass):
    d_xq = nc.dram_tensor("xq", [NQ, DX], F32, kind="ExternalInput").ap()
    d_xk = nc.dram_tensor("xk", [NKV, DX], F32, kind="ExternalInput").ap()
    d_xv = nc.dram_tensor("xv", [NKV, DX], F32, kind="ExternalInput").ap()
    d_tq = nc.dram_tensor("tq", [NQ, DT], F32, kind="ExternalInput").ap()
    d_tk = nc.dram_tensor("tk", [NKV, DT], F32, kind="ExternalInput").ap()
    d_wq = nc.dram_tensor("w_q", [DX, INNER], F32, kind="ExternalInput").ap()
    d_wk = nc.dram_tensor("w_k", [DX, INNER], F32, kind="ExternalInput").ap()
    d_wv = nc.dram_tensor("w_v", [DX, INNER], F32, kind="ExternalInput").ap()
    d_wout = nc.dram_tensor("w_out", [INNER, DX], F32, kind="ExternalInput").ap()
    d_bout = nc.dram_tensor("b_out", [DX], F32, kind="ExternalInput").ap()
    d_kw1 = nc.dram_tensor("kw1", [DT, KHID], F32, kind="ExternalInput").ap()
    d_kb1 = nc.dram_tensor("kb1", [KHID], F32, kind="ExternalInput").ap()
    d_kw2 = nc.dram_tensor("kw2", [KHID, H], F32, kind="ExternalInput").ap()
    d_kb2 = nc.dram_tensor("kb2", [H], F32, kind="ExternalInput").ap()
    d_out = nc.dram_tensor("out", [NQ, DX], F32, kind="ExternalOutput").ap()

    with tile.TileContext(nc) as tc:
        _body(tc, d_xq, d_xk, d_xv, d_tq, d_tk, d_wq, d_wk, d_wv, d_wout,
              d_bout, d_kw1, d_kb1, d_kw2, d_kb2, d_out)
    return nc


def _unit(ap):
    return bass.AP(tensor=ap.tensor, offset=ap.offset, ap=list(ap.ap) + [[1, 1]])


def _col(ap1d):
    return bass.AP(tensor=ap1d.tensor, offset=ap1d.offset,
                   ap=[list(ap1d.ap[0]), [1, 1]])


def _ap(t_ap, offset_elems, dims):
    """Raw AP on the same tensor with explicit [step, num] dims (elems)."""
    return bass.AP(tensor=t_ap.tensor, offset=t_ap.offset + offset_elems,
                   ap=[list(d) for d in dims])


def _fs(t_ap):
    """Free size (elems/partition) of a tile AP = its partition stride."""
    return t_ap.ap[0][0]


def _body(tc, d_xq, d_xk, d_xv, d_tq, d_tk, d_wq, d_wk, d_wv, d_wout,
          d_bout, d_kw1, d_kb1, d_kw2, d_kb2, d_out):
    nc = tc.nc
    ctx = contextlib.ExitStack()
    persist = ctx.enter_context(tc.tile_pool(name="persist", bufs=1))
    dram_pool = ctx.enter_context(tc.tile_pool(name="drsc", bufs=1, space="DRAM"))

    # ================= phase 0: constants & small precompute =================
    ctx0 = contextlib.ExitStack()
    p0 = ctx0.enter_context(tc.tile_pool(name="p0", bufs=1))
    psum0 = ctx0.enter_context(tc.tile_pool(name="psum0", bufs=2, space="PSUM"))

    tqT = p0.tile([DT, NQ], F32)
    tkT = p0.tile([DT, NKV], F32)
    nc.sync.dma_start(out=tqT[:, :], in_=_unit(d_tq.rearrange("q t -> t q")))
    nc.sync.dma_start(out=tkT[:, :], in_=_unit(d_tk.rearrange("k t -> t k")))
    kw1_sb = p0.tile([DT, KHID], F32)
    nc.sync.dma_start(out=kw1_sb[:, :], in_=d_kw1[:, :])
    kb1_sb = p0.tile([KHID, 1], F32)
    nc.sync.dma_start(out=kb1_sb[:, :], in_=_col(d_kb1))
    kw2_sb = p0.tile([KHID, H], F32)
    nc.sync.dma_start(out=kw2_sb[:, :], in_=d_kw2[:, :])
    tqT_bf = p0.tile([DT, NQ], BF16)
    tkT_bf = p0.tile([DT, NKV], BF16)
    kw1_bf = p0.tile([DT, KHID], BF16)
    kw2_bf = p0.tile([KHID, H], BF16)
    nc.vector.tensor_copy(tqT_bf[:, :], tqT[:, :])
    nc.vector.tensor_copy(tkT_bf[:, :], tkT[:, :])
    nc.vector.tensor_copy(kw1_bf[:, :], kw1_sb[:, :])
    nc.vector.tensor_copy(kw2_bf[:, :], kw2_sb[:, :])

    # kb2 pattern: partition p = grp*8 + h  ->  kb2[h]
    kb2_pat = persist.tile([P, 1], F32)
    nc.scalar.dma_start(
        out=kb2_pat[:, :],
        in_=bass.AP(tensor=d_kb2.tensor, offset=d_kb2.offset,
                    ap=[[0, 16], [1, 8]]))

    # aT[c, q] = kw1^T tqT + kb1  (bf16), bounced to DRAM for broadcasts
    aT_ps = psum0.tile([KHID, NQ], F32, tag="aT")
    for j in range(NQ // 512):
        nc.tensor.matmul(aT_ps[:, j * 512:(j + 1) * 512], kw1_bf[:, :],
                         tqT_bf[:, j * 512:(j + 1) * 512], start=True, stop=True)
    aT_bf = p0.tile([KHID, NQ], BF16)
    nc.scalar.activation(aT_bf[:, :], aT_ps[:, :], AF.Identity,
                         bias=kb1_sb[:, :], scale=1.0)
    aT_dram = dram_pool.tile([KHID, NQ], BF16)
    nc.sync.dma_start(out=aT_dram[:, :], in_=aT_bf[:, :])

    # b[k, c] = -(tk kw1); bounce to DRAM, reload as b2[(klo,c), (kt,kg)]
    b_sb = p0.tile([P, NKT, KHID], F32)
    for kt in range(NKT):
        b_ps = psum0.tile([P, KHID], F32, tag="b_ps")
        nc.tensor.matmul(b_ps[:, :], tkT_bf[:, kt * P:(kt + 1) * P],
                         kw1_bf[:, :], start=True, stop=True)
        nc.scalar.activation(b_sb[:, kt, :], b_ps[:, :], AF.Copy, scale=-1.0)
    b_dram = dram_pool.tile([NKV, KHID], F32)
    nc.sync.dma_start(
        out=_ap(b_dram, 0, [[KHID, P], [P * KHID, NKT], [1, KHID]]),
        in_=b_sb[:, :, :])
    b2 = persist.tile([P, NKT * 16], F32)   # [(klo,c), kt*16+kg]
    nc.sync.dma_start(
        out=b2[:, :],
        in_=_ap(b_dram, 0, [[16, 8], [1, 16], [2048, NKT], [128, 16]]))

    # E matrix [128=(klo,c), 64=(klo,h)]: E[klo*16+c, klo*8+h] = kw2[c,h]
    E_sb = persist.tile([P, 64], BF16)
    nc.vector.memset(E_sb[:, :], 0.0)
    for klo in range(8):
        nc.scalar.dma_start(
            out=_unit(E_sb[klo * 16:(klo + 1) * 16, klo * 8:(klo + 1) * 8]),
            in_=_unit(kw2_bf[:, :]))

    bout_bc = persist.tile([P, DX], F32)
    nc.scalar.dma_start(
        out=bout_bc[:, :],
        in_=bass.AP(tensor=d_bout.tensor, offset=d_bout.offset,
                    ap=[[0, P], [1, DX]]))

    # identity [128,128] bf16 via DRAM bounce (diag-strided DMA into DRAM)
    ident = persist.tile([P, P], BF16)
    zeros128 = p0.tile([P, P], BF16)
    ones_col = p0.tile([P, 1], BF16)
    nc.vector.memset(zeros128[:, :], 0.0)
    nc.vector.memset(ones_col[:, :], 1.0)
    ident_dram = dram_pool.tile([P, P], BF16)
    nc.scalar.dma_start(out=ident_dram[:, :], in_=zeros128[:, :])
    nc.scalar.dma_start(
        out=_ap(ident_dram, 0, [[P + 1, P], [1, 1]]),
        in_=ones_col[:, :])
    nc.scalar.dma_start(out=ident[:, :], in_=ident_dram[:, :])


    ctx0.close()

    # ============ main pools (phase 1 + phase 2 share PSUM rings) ============
    ctx2 = contextlib.ExitStack()
    ps_b = ctx2.enter_context(tc.tile_pool(name="ps_b", bufs=2, space="PSUM"))
    ps_d = ctx2.enter_context(tc.tile_pool(name="ps_d", bufs=2, space="PSUM"))
    ps_m = ctx2.enter_context(tc.tile_pool(name="ps_m", bufs=2, space="PSUM"))
    abc_pool = ctx2.enter_context(tc.tile_pool(name="abc", bufs=2))
    r_pool = ctx2.enter_context(tc.tile_pool(name="rp", bufs=3))
    eb_pool = ctx2.enter_context(tc.tile_pool(name="eb", bufs=1))
    ebs_pool = ctx2.enter_context(tc.tile_pool(name="ebs", bufs=2))
    et_pool = ctx2.enter_context(tc.tile_pool(name="et", bufs=3))
    zr_pool = ctx2.enter_context(tc.tile_pool(name="zr", bufs=2))
    ph_pool = ctx2.enter_context(tc.tile_pool(name="ph", bufs=2))
    attn_pool = ctx2.enter_context(tc.tile_pool(name="attn", bufs=2))
    outq_pool = ctx2.enter_context(tc.tile_pool(name="outq", bufs=2))
    o_pool = ctx2.enter_context(tc.tile_pool(name="op", bufs=2))

    def sh_tile():
        return ps_m.tile([P, 512], F32, tag="m", name="m")

    def d_tile():
        return ps_d.tile([P, 2, QC], F32, tag="d", name="d")

    # -------- E_b producer (R -> E-mm -> exp), per (qc, kt) --------
    def produce_start(qc):
        q0 = qc * QC
        A_bc = abc_pool.tile([P, QC], BF16, tag="abc")
        nc.sync.dma_start(
            out=A_bc[:, :],
            in_=_ap(aT_dram, q0, [[0, 8], [NQ, KHID], [1, QC]]))
        E_b = eb_pool.tile([P, 8, NKT, QC], BF16, tag="eb")
        return (A_bc, E_b)

    def produce_chunk(st, kt, use_pool_r=True):
        A_bc, E_b = st
        fs_eb = _fs(E_b)
        for kgh in range(2):
            R_t = r_pool.tile([P, 8, QC], BF16, tag="rt")
            for kg8 in range(8):
                kg = kgh * 8 + kg8
                eng = nc.gpsimd if (use_pool_r and kg8 == 7) else nc.vector
                eng.tensor_scalar(
                    out=R_t[:, kg8, :], in0=A_bc[:, :],
                    scalar1=b2[:, kt * 16 + kg: kt * 16 + kg + 1],
                    scalar2=0.0, op0=AX.add, op1=AX.max)
            bp = ps_b.tile([P, 4, QC], F32, tag="bias")
            for pl in range(4):
                # kg = 2*pair + half ; pair = kgh*4 + pl
                nc.tensor.matmul(bp[0:64, pl, :], E_sb[:, :],
                                 R_t[:, 2 * pl, :], start=True, stop=True)
                nc.tensor.matmul(bp[64:128, pl, :], E_sb[:, :],
                                 R_t[:, 2 * pl + 1, :], start=True,
                                 stop=True, tile_position=(0, 64))
            nc.scalar.activation(
                _ap(E_b, (kgh * 4) * (NKT * QC) + kt * QC,
                    [[fs_eb, P], [NKT * QC, 4], [1, QC]]),
                bp[:, :, :], AF.Exp, bias=kb2_pat[:, :])

    # ================= phase 1: projections =================
    ctx1 = contextlib.ExitStack()
    p1 = ctx1.enter_context(tc.tile_pool(name="p1", bufs=1))

    def load_w(dram, pool, name):
        w = pool.tile([P, NDXT, INNER], BF16, name=name)
        nc.gpsimd.dma_start(out=w[:, :, :],
                            in_=dram.rearrange("(t p) i -> p t i", p=P))
        return w

    wq_raw = load_w(d_wq, p1, "wq")
    # fold the attention SCALE into w_q once (one 4x-mode DVE op)
    wq_bf = p1.tile([P, NDXT, INNER], BF16, name="wqs")
    nc.vector.tensor_scalar(out=wq_bf[:, :, :], in0=wq_raw[:, :, :],
                            scalar1=SCALE, scalar2=0.0,
                            op0=AX.mult, op1=AX.bypass)
    wk_bf = load_w(d_wk, p1, "wk")
    wv_bf = load_w(d_wv, p1, "wv")
    wout_bf = load_w(d_wout, persist, "wout")

    xq_b = p1.tile([P, NQ // P, DX], BF16, name="xq_b")
    nc.gpsimd.dma_start(out=xq_b[:, :, :],
                        in_=d_xq.rearrange("(t p) d -> p t d", p=P))
    xk_b = p1.tile([P, NKT, DX], BF16, name="xk_b")
    xv_b = p1.tile([P, NKT, DX], BF16, name="xv_b")
    for kt in range(NKT):
        # src dims (half, klo, pair, dx) in elems of the f32 DRAM tensor
        src_dims = [[8 * DX, 2], [DX, 8], [16 * DX, 8], [1, DX]]
        nc.gpsimd.dma_start(out=xk_b[:, kt, :],
                            in_=_ap(d_xk, kt * P * DX, src_dims))
        nc.gpsimd.dma_start(out=xv_b[:, kt, :],
                            in_=_ap(d_xv, kt * P * DX, src_dims))

    # chunk-0 E_b production overlaps the x/w loads (R kept off Pool here)
    st = produce_start(0)
    for kt in range(NKT):
        produce_chunk(st, kt, use_pool_r=False)

    # transpose x -> xT [dx-part, 4, 1024] bf16 via PE transpose
    def make_xT(x_b, name):
        xT = p1.tile([P, NDXT, NQ], BF16, name=name)
        for dxt in range(NDXT):
            for qt4 in range(2):
                tpt = sh_tile()
                tpb = tpt.bitcast(BF16)
                fs_tp = _fs(tpb)
                for j in range(4):
                    qt = qt4 * 4 + j
                    nc.tensor.transpose(
                        _ap(tpb, j * P, [[fs_tp, P], [1, P]]),
                        x_b[:, qt, dxt * P:(dxt + 1) * P], ident[:, :])
                nc.vector.tensor_copy(
                    _ap(xT, dxt * NQ + qt4 * 512,
                        [[_fs(xT), P], [P, 4], [1, P]]),
                    _ap(tpb, 0, [[fs_tp, P], [P, 4], [1, P]]))
        return xT

    xqT = make_xT(xq_b, "xqT")
    xkT = make_xT(xk_b, "xkT")
    xvT = make_xT(xv_b, "xvT")

    # qT/kT [128, 4, 1024] bf16 (qT folded with SCALE)
    qT_bf = persist.tile([P, NIT, NQ], BF16)
    kT_bf = persist.tile([P, NIT, NKV], BF16)
    for it in range(NIT):
        for j in range(NQ // 512):
            pq = sh_tile()[:, :]
            pk = sh_tile()[:, :]
            for dt_ in range(NDXT):
                nc.tensor.matmul(pq, wq_bf[:, dt_, it * P:(it + 1) * P],
                                 xqT[:, dt_, j * 512:(j + 1) * 512],
                                 start=(dt_ == 0), stop=(dt_ == NDXT - 1))
            for dt_ in range(NDXT):
                nc.tensor.matmul(pk, wk_bf[:, dt_, it * P:(it + 1) * P],
                                 xkT[:, dt_, j * 512:(j + 1) * 512],
                                 start=(dt_ == 0), stop=(dt_ == NDXT - 1))
            nc.vector.tensor_copy(qT_bf[:, it, j * 512:(j + 1) * 512], pq)
            nc.scalar.activation(kT_bf[:, it, j * 512:(j + 1) * 512], pk,
                                 AF.Copy)
    # fold SCALE into qT via DVE in-place? simpler: scale at dots time is not
    # possible; instead fold SCALE into qT by scaling the copy on ACT:
    # (gpsimd copy above is unscaled for kT; qT needs SCALE -> redo on ACT)

    # v5 [128, kt, h, 65] bf16 = [v_h | ones]
    v5 = persist.tile([P, NKT, H, 65], BF16)
    for kt in range(NKT):
        pp = sh_tile()
        pv = pp[:, :]
        for dt_ in range(NDXT):
            nc.tensor.matmul(pv, xvT[:, dt_, kt * P:(kt + 1) * P],
                             wv_bf[:, dt_, :],
                             start=(dt_ == 0), stop=(dt_ == NDXT - 1))
        nc.vector.tensor_copy(
            _ap(v5, kt * (H * 65), [[_fs(v5), P], [65, H], [1, 64]]),
            _ap(pp, 0, [[_fs(pp), P], [64, H], [1, 64]]))
    nc.gpsimd.memset(
        _ap(v5, 64, [[_fs(v5), P], [H * 65, NKT], [65, H], [1, 1]]), 1.0)

    ctx1.close()

    # ================= phase 2: attention per q-chunk =================
    def consume_head(qc, h, Ebs, attn):
        q0 = qc * QC
        it = h // 2
        r0 = (h % 2) * 64
        hh = h % 4
        P_h = ph_pool.tile([P, NKT, QC], BF16, tag="ph")
        for kth in range(4):
            dpt = d_tile()
            for j in range(2):
                kt = kth * 2 + j
                nc.tensor.matmul(
                    dpt[:, j, :],
                    kT_bf[r0:r0 + 64, it, kt * P:(kt + 1) * P],
                    qT_bf[r0:r0 + 64, it, q0:q0 + QC],
                    start=True, stop=True)
            E_t = et_pool.tile([P, 2, QC], BF16, tag="et")
            nc.scalar.activation(E_t[:, :, :], dpt[:, :, :], AF.Exp)
            nc.vector.tensor_mul(
                P_h[:, kth * 2:(kth + 1) * 2, :],
                E_t[:, :, :],
                Ebs[:, hh, kth * 2:(kth + 1) * 2, :])
        for qt2 in range(QC // P):
            avt = sh_tile()
            av = _ap(avt, 0, [[_fs(avt), P], [1, 65]])
            for kt in range(NKT):
                nc.tensor.matmul(
                    av,
                    P_h[:, kt, qt2 * P:(qt2 + 1) * P],
                    v5[:, kt, h, :],
                    start=(kt == 0), stop=(kt == NKT - 1))
            zr = zr_pool.tile([P, 1], F32, tag="zr")
            nc.vector.reciprocal(zr[:, :], _ap(avt, 64, [[_fs(avt), P], [1, 1]]))
            nc.vector.tensor_scalar(
                out=attn[:, qt2, h, :],
                in0=_ap(avt, 0, [[_fs(avt), P], [1, 64]]),
                scalar1=zr[:, :], scalar2=0.0,
                op0=AX.mult, op1=AX.bypass)

    def tail(qc, attn):
        outT = outq_pool.tile([P, NIT, QC], BF16, tag="outT")
        fs_attn = _fs(attn)
        fs_outT = _fs(outT)
        for qt2 in range(QC // P):
            tpt = sh_tile()
            tpb = tpt.bitcast(BF16)
            fs_tp = _fs(tpb)
            for it in range(NIT):
                nc.tensor.transpose(
                    _ap(tpb, it * P, [[fs_tp, P], [1, P]]),
                    _ap(attn, qt2 * (H * 64) + it * 128,
                        [[fs_attn, P], [1, P]]),
                    ident[:, :])
            nc.vector.tensor_copy(
                _ap(outT, qt2 * P, [[fs_outT, P], [QC, NIT], [1, P]]),
                _ap(tpb, 0, [[fs_tp, P], [P, 4], [1, P]]))
        for qt2 in range(QC // P):
            opt = sh_tile()
            op = _ap(opt, 0, [[_fs(opt), P], [1, DX]])
            for it in range(NIT):
                nc.tensor.matmul(op,
                                 outT[:, it, qt2 * P:(qt2 + 1) * P],
                                 wout_bf[:, it, :],
                                 start=(it == 0), stop=(it == NIT - 1))
            o_sb = o_pool.tile([P, DX], F32, tag="osb")
            nc.vector.tensor_add(o_sb[:, :], op, bout_bc[:, :])
            qt_glob = qc * (QC // P) + qt2
            nc.sync.dma_start(out=d_out[qt_glob * P:(qt_glob + 1) * P, :],
                              in_=o_sb[:, :])

    for qc in range(NQC):
        A_bc, E_b = st
        fs_eb = _fs(E_b)
        if qc + 1 < NQC:
            st = produce_start(qc + 1)
        # shuffle exp(bias) into per-head [k~, q] layout: Ebs [k~, h4, kt, q]
        ebs_tiles = []
        for hgrp in range(2):
            Ebs = ebs_pool.tile([P, 4, NKT, QC], BF16, tag="ebs")
            fs_ebs = _fs(Ebs)
            for half in range(2):
                for hh in range(4):
                    h = hgrp * 4 + hh
                    dma_eng = nc.sync if half == 0 else nc.gpsimd
                    dma_eng.dma_start(
                        out=_ap(Ebs, half * 64 * fs_ebs + hh * (NKT * QC),
                                [[fs_ebs, 64], [QC, NKT], [1, QC]]),
                        in_=_ap(E_b, (half * 64 + h) * fs_eb,
                                [[8 * fs_eb, 8], [NKT * QC, 8],
                                 [QC, NKT], [1, QC]]))
            ebs_tiles.append(Ebs)

        attn = attn_pool.tile([P, QC // P, H, 64], BF16, tag="attn")
        for i in range(8):
            if qc + 1 < NQC:
                produce_chunk(st, i)
            consume_head(qc, i, ebs_tiles[i // 4], attn)
        tail(qc, attn)

    ctx2.close()
    ctx.close()


_NC_CACHE = None


def _get_nc():
    global _NC_CACHE
    if _NC_CACHE is None:
        nc = bacc.Bacc("TRN2", target_bir_lowering=False, debug=False,
                       enable_asserts=False, num_devices=M)
        build_kernel(nc)
        nc.compile()
        _NC_CACHE = nc
    return _NC_CACHE


def kernel(**inputs):
    nc = _get_nc()
    shared = {n: np.ascontiguousarray(np.asarray(inputs[n], dtype=np.float32))
              for n in ["w_q", "w_k", "w_v", "w_out", "b_out",
                        "kw1", "kb1", "kw2", "kb2"]}
    in_maps = []
    for i in range(M):
        m = dict(shared)
        for n in ["xq", "xk", "xv", "tq", "tk"]:
            m[n] = np.ascontiguousarray(np.asarray(inputs[n][i], dtype=np.float32))
        in_maps.append(m)
    res = bass_utils.run_bass_kernel_spmd(nc, in_maps, core_ids=list(range(M)))
    out = np.stack([res.results[i]["out"] for i in range(M)], axis=0)
    return out.astype(np.float32)


if __name__ == "__main__":
    import reference
    inputs = {k: np.asarray(v) for k, v in reference.setup_inputs().items()}
    out = kernel(**inputs)
    print("out", out.shape, out.dtype)
